# revision 16
# baseline (speedup 1.0000x reference)
"""Trainium2 Bass kernel for nn_DecoderBlock (B=2, N=2048, M=1024, D=1024, H=16).

Sharding: sequence-parallel across 8 cores. Core c handles batch b=c//4,
row chunk a=c%4 (512 rows). K/V for self-attention and cross-attention are
computed on the owning core and AllGathered within each 4-core batch group
(replica groups [[0-3],[4-7]] make the gathered layout group-relative, so the
same program works on every core).

Causality: strict-past key blocks are masked with a per-key additive bias
fused into the softmax exp (bias = 0 or -1e30, per-core data); the "own rows"
diagonal uses an in-SBUF pass with a compile-time triangle mask.

Matmul dtypes: float32r (tf32-like, full PE rate) for projections and scores;
bf16 for the post-softmax side (attn weights, V, W_o, W2).
"""
import numpy as np

import concourse.bass as bass
import concourse.mybir as mybir
import concourse.tile as tile
from concourse import bacc
from concourse.masks import make_identity

P = 128
D = 1024
DFF = 4096
NQ = 512          # rows per core
MQ = 256          # memory rows per core
H = 16
NEG = -1e30
F32 = mybir.dt.float32
F32R = mybir.dt.float32r
BF16 = mybir.dt.bfloat16
AF = mybir.ActivationFunctionType
ALU = mybir.AluOpType
EXP_SCALE = 0.125  # 1/sqrt(64)

WAVES = [(0, 6), (6, 6), (12, 4)]


def build_nc(reps=1, debug=False):
    nc = bacc.Bacc("TRN2", target_bir_lowering=False, debug=False, num_devices=8)

    def din(name, shape, dt=F32):
        return nc.dram_tensor(name, shape, dt, kind="ExternalInput")

    xT_d = din("xT", [D, NQ], F32R)
    xow_d = din("xow", [NQ, D])
    memT_d = din("memT", [D, MQ], F32R)
    w = {}
    for pre in ("sa", "ca"):
        for nm in ("q", "k", "v", "o"):
            wdt = BF16 if nm == "o" else F32R
            w[f"{pre}_w{nm}"] = din(f"{pre}_w{nm}", [D, D], wdt)
            bshape = [P, 8] if nm in ("q", "k") else [D]
            w[f"{pre}_b{nm}"] = din(f"{pre}_b{nm}", bshape)
    w1_d = din("w1", [D, DFF], F32R)
    b1_d = din("b1", [P, 32])
    w2_d = din("w2", [DFF, D], BF16)
    b2_d = din("b2", [D])
    lng = {}
    for j in (1, 2, 3):
        lng[f"g{j}"] = din(f"g{j}", [D])
        lng[f"b{j}"] = din(f"lnb{j}", [D])
    tri_d = din("tri", [P, P])
    kbias_d = din("kbias", [P, 12])
    out_d = nc.dram_tensor("out", [NQ, D], F32, kind="ExternalOutput")
    dbg = {}
    if debug:
        dbg["dbg_qT"] = nc.dram_tensor("dbg_qT", [D, NQ], F32R,
                                       kind="ExternalOutput")
        dbg["dbg_sums"] = nc.dram_tensor("dbg_sums", [H, NQ], F32,
                                         kind="ExternalOutput")
        dbg["dbg_ex0"] = nc.dram_tensor("dbg_ex0", [P, NQ], F32R,
                                        kind="ExternalOutput")
        for nm in ("dbg_x1", "dbg_x1n", "dbg_x2n"):
            dbg[nm] = nc.dram_tensor(nm, [NQ, D], F32, kind="ExternalOutput")

    with tile.TileContext(nc) as tc:
        with (
            tc.tile_pool(name="pp", bufs=1) as pp,
            tc.tile_pool(name="st", bufs=3) as st,
            tc.tile_pool(name="st2", bufs=2) as st2,
            tc.tile_pool(name="ws", bufs=3) as ws,
            tc.tile_pool(name="sc", bufs=2, space="PSUM") as sc,
            tc.tile_pool(name="acc", bufs=1, space="PSUM") as acc_pool,
            tc.tile_pool(name="dram", bufs=1, space="DRAM") as dram,
        ):
            # ---------------- small constants ----------------
            idf = pp.tile([P, P], F32, tag="idf")
            make_identity(nc, idf)
            eps_sb = pp.tile([P, 1], F32, tag="eps")
            nc.vector.memset(eps_sb[:], 1e-5)
            z_sb = pp.tile([P, 1], F32, tag="zeros")
            nc.vector.memset(z_sb[:], 0.0)
            tri_sb = pp.tile([P, P], F32, tag="tri")
            nc.sync.dma_start(tri_sb[:], tri_d[:])
            kbias_sb = pp.tile([P, 12], F32, tag="kbias")
            nc.sync.dma_start(kbias_sb[:], kbias_d[:])

            def col_bias(name, dram_t):
                t = pp.tile(list(dram_t.shape), F32, tag=name, name=name)
                nc.sync.dma_start(t[:], dram_t[:])
                return t

            bq_sa = col_bias("bq_sa", w["sa_bq"])
            bk_sa = col_bias("bk_sa", w["sa_bk"])
            bq_ca = col_bias("bq_ca", w["ca_bq"])
            bk_ca = col_bias("bk_ca", w["ca_bk"])
            b1_t = col_bias("b1_t", b1_d)

            def row_bcast(dram_t, nm):
                """[D] vector broadcast to [128, D] (rotating slots)."""
                t = st2.tile([P, D], F32, tag="rowb", name=f"rowb_{nm}")
                nc.gpsimd.dma_start(t[:], dram_t[None, :].to_broadcast([P, D]))
                return t

            # ---------------- helpers ----------------
            def proj_T(w_dram, bias_t, rhs_sb, out_sb, n_free):
                """out_sb[:, ob, :] = W[:, ob].T @ rhs + b (o on partitions)."""
                for ob in range(8):
                    ps = sc.tile([P, 512], F32, tag="sc", name="ps_proj")
                    for i in range(8):
                        wt = ws.tile([P, P], F32R, tag=f"wblk{i % 4}", name="wt")
                        nc.sync.dma_start(
                            wt[:], w_dram[i * P:(i + 1) * P, ob * P:(ob + 1) * P])
                        nc.tensor.matmul(ps[:, :n_free], wt[:], rhs_sb[:, i, :],
                                         start=(i == 0), stop=(i == 7))
                    nc.vector.tensor_scalar(out_sb[:, ob, :], ps[:, :n_free],
                                            bias_t[:, ob:ob + 1], None, ALU.add)

            def v_proj(w_dram, bvb, rhs_sb, nblk, vo_sb):
                """vo_sb[:, mb, h, :64] = (rhs.T W_v + b_v) per head; col 64 = 1."""
                nc.vector.memset(vo_sb[:, :, :, 64].bitcast(F32), 1.0)
                for mb in range(nblk):
                    for oh in range(2):
                        ps = sc.tile([P, 512], F32, tag="sc", name="ps_v")
                        for i in range(8):
                            wt = ws.tile([P, 512], F32R, tag=f"wrhs{i % 2}",
                                         name="wtv")
                            nc.sync.dma_start(
                                wt[:], w_dram[i * P:(i + 1) * P,
                                              oh * 512:(oh + 1) * 512])
                            nc.tensor.matmul(ps[:],
                                             rhs_sb[:, i, mb * P:(mb + 1) * P],
                                             wt[:], start=(i == 0), stop=(i == 7))
                        for hh in range(8):
                            h = oh * 8 + hh
                            nc.vector.tensor_tensor(
                                vo_sb[:, mb, h, 0:64],
                                ps[:, hh * 64:(hh + 1) * 64],
                                bvb[:, h * 64:h * 64 + 64], ALU.add)

            def attention(qT_sb, ag_k, ag_vo, n_kb, kb_to_src, attnT_sb,
                          kb_bias=None, own=None, name=""):
                dbg_sa = debug and name == "sa"
                for (w0, wh) in WAVES:
                    accs = {}
                    for h in range(w0, w0 + wh):
                        accs[h] = acc_pool.tile([65, 512], F32,
                                                tag=f"acc{h - w0}",
                                                name=f"acc_{name}_{h}")
                    for kb in range(n_kb):
                        krow0, kcol0, vrow0 = kb_to_src(kb)
                        kts = {}
                        for hp in range(w0 // 2, (w0 + wh + 1) // 2):
                            kt = st.tile([P, P], F32R, tag=f"kt{hp % 3}",
                                         name=f"kt_{name}_{hp}")
                            nc.sync.dma_start(
                                kt[:], ag_k[krow0 + hp * P:krow0 + (hp + 1) * P,
                                            kcol0:kcol0 + P])
                            kts[hp] = kt
                        vot = st.tile([P, wh * 65], F32R, tag="vot",
                                      name=f"vot_{name}")
                        nc.sync.dma_start(
                            vot[:], ag_vo[vrow0:vrow0 + P,
                                          w0 * 65:(w0 + wh) * 65])
                        for h in range(w0, w0 + wh):
                            hp, hs = h // 2, h % 2
                            scs = sc.tile([P, 512], F32, tag="sc", name="ps_sc")
                            nc.tensor.matmul(
                                scs[:], kts[hp][hs * 64:(hs + 1) * 64, :],
                                qT_sb[hs * 64:(hs + 1) * 64, hp, :],
                                start=True, stop=True)
                            ex = st2.tile([P, 512], F32R, tag="ex", name="ex")
                            bias = (kb_bias[:, kb:kb + 1]
                                    if kb_bias is not None else z_sb[:])
                            nc.scalar.activation(ex[:], scs[:], AF.Exp,
                                                 bias=bias, scale=EXP_SCALE)
                            nc.tensor.matmul(
                                accs[h][:],
                                vot[:, (h - w0) * 65:(h - w0 + 1) * 65], ex[:],
                                start=(kb == 0),
                                stop=(own is None and kb == n_kb - 1))
                    if own is not None:
                        kT_own, vo_own = own
                        for h in range(w0, w0 + wh):
                            hp, hs = h // 2, h % 2
                            for jj in range(4):
                                qn = 512 - jj * P
                                scs = sc.tile([P, 512], F32, tag="sc",
                                              name="ps_own")
                                nc.tensor.matmul(
                                    scs[:, :qn],
                                    kT_own[hs * 64:(hs + 1) * 64, hp,
                                           jj * P:(jj + 1) * P],
                                    qT_sb[hs * 64:(hs + 1) * 64, hp, jj * P:512],
                                    start=True, stop=True)
                                nc.vector.tensor_tensor(scs[:, :P], scs[:, :P],
                                                        tri_sb[:], ALU.add)
                                ex = st2.tile([P, 512], F32R, tag="ex", name="ex2")
                                nc.scalar.activation(ex[:, :qn], scs[:, :qn],
                                                     AF.Exp, bias=z_sb[:],
                                                     scale=EXP_SCALE)
                                if dbg_sa and h == 0 and jj == 0:
                                    nc.sync.dma_start(dbg["dbg_ex0"][:], ex[:])
                                nc.tensor.matmul(accs[h][:, jj * P:512],
                                                 vo_own[:, jj, h, :], ex[:, :qn],
                                                 start=False, stop=(jj == 3))
                    for h in range(w0, w0 + wh):
                        hp, hs = h // 2, h % 2
                        rec = st2.tile([1, 512], F32, tag="rec", name="rec")
                        nc.vector.reciprocal(rec[:], accs[h][64:65, :])
                        if dbg_sa:
                            smt = st2.tile([1, 512], F32, tag="rec",
                                           name="smt")
                            nc.vector.tensor_copy(smt[:], accs[h][64:65, :])
                            nc.sync.dma_start(dbg["dbg_sums"][h:h + 1, :],
                                              smt[:])
                        recb = st2.tile([64, 512], F32, tag="recb", name="recb")
                        nc.gpsimd.partition_broadcast(recb[:], rec[:])
                        nc.vector.tensor_tensor(
                            attnT_sb[hs * 64:(hs + 1) * 64, hp, :],
                            accs[h][0:64, :], recb[:], ALU.mult)

            def o_proj(w_dram, attnT_sb, resid_sb, bo_vec, x_out):
                """x_out = attnT.T @ Wo + resid + bo  ([128, 4, 1024] f32)."""
                bob = row_bcast(bo_vec, "bo")
                for rb in range(4):
                    for oh in range(2):
                        ps = sc.tile([P, 512], F32, tag="sc", name="ps_o")
                        for db in range(8):
                            wt = ws.tile([P, 512], BF16, tag=f"wrhsb{db % 2}",
                                         name="wto")
                            nc.sync.dma_start(
                                wt[:], w_dram[db * P:(db + 1) * P,
                                              oh * 512:(oh + 1) * 512])
                            nc.tensor.matmul(
                                ps[:], attnT_sb[:, db, rb * P:(rb + 1) * P],
                                wt[:], start=(db == 0), stop=(db == 7))
                        sl = slice(oh * 512, (oh + 1) * 512)
                        nc.vector.tensor_tensor(x_out[:, rb, sl], ps[:],
                                                resid_sb[:, rb, sl], ALU.add)
                        nc.vector.tensor_tensor(x_out[:, rb, sl],
                                                x_out[:, rb, sl],
                                                bob[:, sl], ALU.add)

            def layernorm(x_sb, out_sb, g_vec, b_vec):
                gb = row_bcast(g_vec, "g")
                bb = row_bcast(b_vec, "b")
                for rb in range(4):
                    x_row, out_row = x_sb[:, rb, :], out_sb[:, rb, :]
                    negmu = st.tile([P, 1], F32, tag="negmu", name="negmu")
                    nc.vector.reduce_sum(negmu[:], x_row,
                                         axis=mybir.AxisListType.X)
                    nc.vector.tensor_scalar_mul(negmu[:], negmu[:], -1.0 / D)
                    cent = st2.tile([P, D], F32, tag="lncent", name="cent")
                    nc.vector.tensor_scalar(cent[:], x_row, negmu[:], None,
                                            ALU.add)
                    ssq = st.tile([P, 1], F32, tag="lnssq", name="ssq")
                    sq = st2.tile([P, D], F32, tag="lncent", name="sq")
                    nc.scalar.activation(sq[:], cent[:], AF.Square,
                                         accum_out=ssq[:])
                    std = st.tile([P, 1], F32, tag="lnstd", name="std")
                    nc.scalar.activation(std[:], ssq[:], AF.Sqrt,
                                         bias=eps_sb[:], scale=1.0 / D)
                    rstd = st.tile([P, 1], F32, tag="lnrstd", name="rstd")
                    nc.vector.reciprocal(rstd[:], std[:])
                    nc.vector.tensor_scalar(out_row, cent[:], rstd[:], None,
                                            ALU.mult)
                    nc.vector.tensor_tensor(out_row, out_row, gb[:], ALU.mult)
                    nc.vector.tensor_tensor(out_row, out_row, bb[:], ALU.add)

            def transpose_to(x_sb, out_sb):
                """[128, 4, 1024] f32 (rows, D) -> [128, 8, 512] f32r (D, rows)."""
                for rb in range(4):
                    for db in range(8):
                        pst = sc.tile([P, 512], F32, tag="sc", name="ps_tr")
                        nc.tensor.transpose(pst[:, :P],
                                            x_sb[:, rb, db * P:(db + 1) * P],
                                            idf[:])
                        nc.vector.tensor_copy(out_sb[:, db, rb * P:(rb + 1) * P],
                                              pst[:, :P])

            groups = [[0, 1, 2, 3], [4, 5, 6, 7]]

            def allgather(src, dst):
                nc.gpsimd.collective_compute(
                    "AllGather", ALU.bypass, replica_groups=groups,
                    ins=[src.opt()], outs=[dst.opt()])

            # ---------------- load inputs ----------------
            xT_sb = pp.tile([P, 8, NQ], F32R, tag="xT", name="xT_sb")
            nc.sync.dma_start(xT_sb[:], xT_d.rearrange("(i p) r -> p i r", p=P))
            xow_sb = pp.tile([P, 4, D], F32, tag="xowin", name="xow_sb")
            nc.sync.dma_start(xow_sb[:], xow_d.rearrange("(j p) d -> p j d", p=P))
            memT_sb = pp.tile([P, 8, MQ], F32R, tag="vo", name="memT_sb")
            nc.sync.dma_start(memT_sb[:], memT_d.rearrange("(i p) m -> p i m", p=P))

            # ------------- CA K/V early (AG overlaps SA work) -------------
            ckT_sb = pp.tile([P, 8, MQ], F32R, tag="ck", name="ckT_sb")
            proj_T(w["ca_wk"], bk_ca, memT_sb, ckT_sb, MQ)
            ag_ck_in = dram.tile([D, MQ], F32R)
            nc.sync.dma_start(ag_ck_in.rearrange("(o p) m -> p o m", p=P),
                              ckT_sb[:])
            ag_ck = dram.tile([4 * D, MQ], F32R)
            allgather(ag_ck_in, ag_ck)

            cvo_sb = pp.tile([P, 2, H, 65], F32R, tag="cvo", name="cvo_sb")
            bv_ca_b = row_bcast(w["ca_bv"], "bvca")
            v_proj(w["ca_wv"], bv_ca_b, memT_sb, 2, cvo_sb)
            ag_cvo_in = dram.tile([MQ, H * 65], F32R)
            nc.sync.dma_start(ag_cvo_in.rearrange("(m p) f -> p m f", p=P),
                              cvo_sb[:].rearrange("p m h e -> p m (h e)"))
            ag_cvo = dram.tile([4 * MQ, H * 65], F32R)
            allgather(ag_cvo_in, ag_cvo)

            # ---------------- main block ----------------
            for rep in range(reps):
                # SA projections
                qT_sb = pp.tile([P, 8, NQ], F32R, tag="q", name="qT_sb")
                proj_T(w["sa_wq"], bq_sa, xT_sb, qT_sb, NQ)
                kT_sb = pp.tile([P, 8, NQ], F32R, tag="kT", name="kT_sb")
                proj_T(w["sa_wk"], bk_sa, xT_sb, kT_sb, NQ)
                ag_k_in = dram.tile([D, NQ], F32R)
                nc.sync.dma_start(ag_k_in.rearrange("(o p) r -> p o r", p=P),
                                  kT_sb[:])
                ag_k = dram.tile([4 * D, NQ], F32R)
                allgather(ag_k_in, ag_k)

                vo_sb = pp.tile([P, 4, H, 65], F32R, tag="vo", name="vo_sb")
                bv_sa_b = row_bcast(w["sa_bv"], "bvsa")
                v_proj(w["sa_wv"], bv_sa_b, xT_sb, 4, vo_sb)
                ag_vo_in = dram.tile([NQ, H * 65], F32R)
                nc.sync.dma_start(ag_vo_in.rearrange("(j p) f -> p j f", p=P),
                                  vo_sb[:].rearrange("p j h e -> p j (h e)"))
                ag_vo = dram.tile([4 * NQ, H * 65], F32R)
                allgather(ag_vo_in, ag_vo)

                # SA attention
                attnT_sb = pp.tile([P, 8, NQ], BF16, tag="attnT", name="attnT_sb")
                attention(qT_sb, ag_k, ag_vo, 12,
                          lambda kb: ((kb // 4) * D, (kb % 4) * P,
                                      (kb // 4) * NQ + (kb % 4) * P),
                          attnT_sb, kb_bias=kbias_sb, own=(kT_sb, vo_sb),
                          name="sa")

                # SA out proj + residual + ln1
                x1_sb = pp.tile([P, 4, D], F32, tag="xA", name="x1_sb")
                o_proj(w["sa_wo"], attnT_sb, xow_sb, w["sa_bo"], x1_sb)
                x1n_sb = pp.tile([P, 4, D], F32, tag="kT", name="x1n_sb")
                layernorm(x1_sb, x1n_sb, lng["g1"], lng["b1"])
                x1nT_sb = pp.tile([P, 8, NQ], F32R, tag="xnT", name="x1nT_sb")
                transpose_to(x1n_sb, x1nT_sb)

                # CA
                qT2_sb = pp.tile([P, 8, NQ], F32R, tag="xT", name="qT2_sb")
                proj_T(w["ca_wq"], bq_ca, x1nT_sb, qT2_sb, NQ)
                attnT2_sb = pp.tile([P, 8, NQ], BF16, tag="attnT",
                                    name="attnT2_sb")
                attention(qT2_sb, ag_ck, ag_cvo, 8,
                          lambda kb: ((kb // 2) * D, (kb % 2) * P,
                                      (kb // 2) * MQ + (kb % 2) * P),
                          attnT2_sb, name="ca")

                x2_sb = pp.tile([P, 4, D], F32, tag="xA", name="x2_sb")
                o_proj(w["ca_wo"], attnT2_sb, x1n_sb, w["ca_bo"], x2_sb)
                x2n_sb = pp.tile([P, 4, D], F32, tag="kT", name="x2n_sb")
                layernorm(x2_sb, x2n_sb, lng["g2"], lng["b2"])
                x2nT_sb = pp.tile([P, 8, NQ], F32R, tag="q", name="x2nT_sb")
                transpose_to(x2n_sb, x2nT_sb)

                # FFN, hidden in 4 chunks of 1024
                x3_sb = pp.tile([P, 4, D], F32, tag="xA", name="x3_sb")
                for hc in range(4):
                    hTc = pp.tile([P, 8, NQ], BF16, tag="hTc", name="hTc")
                    for jb in range(8):
                        jg = hc * 8 + jb
                        ps = sc.tile([P, 512], F32, tag="sc", name="ps_h")
                        for i in range(8):
                            wt = ws.tile([P, P], F32R, tag=f"wblk{i % 4}",
                                         name="wt1")
                            nc.sync.dma_start(
                                wt[:], w1_d[i * P:(i + 1) * P,
                                            jg * P:(jg + 1) * P])
                            nc.tensor.matmul(ps[:], wt[:], x2nT_sb[:, i, :],
                                             start=(i == 0), stop=(i == 7))
                        nc.scalar.activation(hTc[:, jb, :], ps[:], AF.Relu,
                                             bias=b1_t[:, jg:jg + 1], scale=1.0)
                    for rb in range(4):
                        for oh in range(2):
                            ps = sc.tile([P, 512], F32, tag="sc", name="ps_y")
                            for jb in range(8):
                                jg = hc * 8 + jb
                                wt = ws.tile([P, 512], BF16,
                                             tag=f"wrhsb{jb % 2}", name="wt2")
                                nc.sync.dma_start(
                                    wt[:], w2_d[jg * P:(jg + 1) * P,
                                                oh * 512:(oh + 1) * 512])
                                nc.tensor.matmul(
                                    ps[:], hTc[:, jb, rb * P:(rb + 1) * P],
                                    wt[:], start=(jb == 0), stop=(jb == 7))
                            sl = slice(oh * 512, (oh + 1) * 512)
                            if hc == 0:
                                nc.vector.tensor_copy(x3_sb[:, rb, sl], ps[:])
                            else:
                                nc.vector.tensor_tensor(x3_sb[:, rb, sl],
                                                        x3_sb[:, rb, sl],
                                                        ps[:], ALU.add)
                b2b = row_bcast(b2_d, "b2")
                for rb in range(4):
                    nc.vector.tensor_tensor(x3_sb[:, rb, :], x3_sb[:, rb, :],
                                            x2n_sb[:, rb, :], ALU.add)
                    nc.vector.tensor_tensor(x3_sb[:, rb, :], x3_sb[:, rb, :],
                                            b2b[:], ALU.add)
                out_sb = pp.tile([P, 4, D], F32, tag="xnT", name="out_sb")
                layernorm(x3_sb, out_sb, lng["g3"], lng["b3"])

                if rep < reps - 1:
                    xow_sb = out_sb
                    nxT = pp.tile([P, 8, NQ], F32R, tag="xT", name="nxT")
                    transpose_to(out_sb, nxT)
                    xT_sb = nxT

            nc.sync.dma_start(out_d.rearrange("(j p) d -> p j d", p=P), out_sb[:])

            if debug:
                def dump_T(t, name):
                    nc.sync.dma_start(
                        dbg[name].rearrange("(o p) r -> p o r", p=P), t[:])

                def dump_R(t, name):
                    nc.sync.dma_start(
                        dbg[name].rearrange("(j p) d -> p j d", p=P), t[:])

                dump_T(qT_sb, "dbg_qT")
                dump_R(x1_sb, "dbg_x1")
                dump_R(x1n_sb, "dbg_x1n")
                dump_R(x2n_sb, "dbg_x2n")

    nc.compile()
    return nc


# ----------------------------------------------------------------------------
# Host side
# ----------------------------------------------------------------------------
_NC_CACHE = {}


def _get_nc(reps=1, debug=False):
    key = (reps, debug)
    if key not in _NC_CACHE:
        _NC_CACHE[key] = build_nc(reps, debug)
    return _NC_CACHE[key]


def make_in_maps(x, memory, mask,
                 sa_wq, sa_bq, sa_wk, sa_bk, sa_wv, sa_bv, sa_wo, sa_bo,
                 ca_wq, ca_bq, ca_wk, ca_bk, ca_wv, ca_bv, ca_wo, ca_bo,
                 ff_w1, ff_b1, ff_w2, ff_b2,
                 ln1_g, ln1_b, ln2_g, ln2_b, ln3_g, ln3_b):
    import ml_dtypes
    x = np.asarray(x, np.float32)
    memory = np.asarray(memory, np.float32)
    mask2d = np.asarray(mask).reshape(x.shape[1], x.shape[1])

    f32c = lambda a: np.ascontiguousarray(np.asarray(a, np.float32))
    bf16t = lambda a: np.ascontiguousarray(
        np.asarray(a, np.float32).T.astype(ml_dtypes.bfloat16))
    colb = lambda a: np.ascontiguousarray(
        np.asarray(a, np.float32).reshape(-1, P).T)
    tr = lambda a: np.ascontiguousarray(np.asarray(a, np.float32).T)

    blk = mask2d[0:P, 0:P]
    tri = np.where(blk.T != 0, 0.0, NEG).astype(np.float32)

    common = {
        "sa_wq": tr(sa_wq), "sa_bq": colb(sa_bq),
        "sa_wk": tr(sa_wk), "sa_bk": colb(sa_bk),
        "sa_wv": tr(sa_wv), "sa_bv": f32c(sa_bv),
        "sa_wo": bf16t(sa_wo), "sa_bo": f32c(sa_bo),
        "ca_wq": tr(ca_wq), "ca_bq": colb(ca_bq),
        "ca_wk": tr(ca_wk), "ca_bk": colb(ca_bk),
        "ca_wv": tr(ca_wv), "ca_bv": f32c(ca_bv),
        "ca_wo": bf16t(ca_wo), "ca_bo": f32c(ca_bo),
        "w1": tr(ff_w1), "b1": colb(ff_b1),
        "w2": bf16t(ff_w2), "b2": f32c(ff_b2),
        "g1": f32c(ln1_g), "lnb1": f32c(ln1_b),
        "g2": f32c(ln2_g), "lnb2": f32c(ln2_b),
        "g3": f32c(ln3_g), "lnb3": f32c(ln3_b),
        "tri": tri,
    }

    in_maps = []
    for c in range(8):
        b, a = c // 4, c % 4
        rows = slice(a * NQ, (a + 1) * NQ)
        mrows = slice(a * MQ, (a + 1) * MQ)
        xb = x[b]
        kbias = np.empty((P, 12), np.float32)
        msub = mask2d[rows]
        own_start = a * NQ
        for kb in range(12):
            vis = msub[:, kb * P:(kb + 1) * P].all(axis=0)
            not_own = (kb * P + np.arange(P)) < own_start
            kbias[:, kb] = np.where(vis & not_own, 0.0, NEG)
        m = dict(common)
        m["xT"] = f32c(xb.T[:, rows])
        m["xow"] = f32c(xb[rows])
        m["memT"] = f32c(memory[b].T[:, mrows])
        m["kbias"] = kbias
        in_maps.append(m)
    return in_maps


def kernel(**inputs):
    from concourse.bass_utils import run_bass_kernel_spmd
    nc = _get_nc(1, False)
    in_maps = make_in_maps(**inputs)
    res = run_bass_kernel_spmd(nc, in_maps, list(range(8)))
    x = np.asarray(inputs["x"])
    B, N, _ = x.shape
    out = np.empty((B, N, D), np.float32)
    for c in range(8):
        b, a = c // 4, c % 4
        out[b, a * NQ:(a + 1) * NQ] = res.results[c]["out"]
    return out
